# revision 15
# baseline (speedup 1.0000x reference)
"""Multi-head self-attention on 8 Trainium2 NeuronCores.

Strategy (tensor parallel over heads, per the classic Megatron split):
  - 16 heads / 8 cores -> each core owns 2 heads (a 128-column slice of
    Wq/Wk/Wv and the matching 128-row slice of Wo).
  - x is transposed on the host to xT [D, B*S] and replicated to every
    core; each core projects QT/KT/VT for its heads, runs attention for
    its (batch, head) pairs, and produces a partial output projection
    [B*S, D].
  - Host sums the 8 partials (the Wo row-parallel all-reduce) and adds bo.

Per-core kernel layout notes:
  - All matmuls run in float32r (full PE rate at free-dim >= 256,
    ~1.5e-4 rel rms per matmul vs fp32).
  - Scores are computed transposed, ST[k, q] = KT.T @ QT, two heads
    row-packed into the PE array (contraction is only 64 wide per head).
  - softmax denominator rides the attention matmul: V is augmented with
    a ones column, so AV psum row 64 is sum_k exp(s).
  - exp happens on ACT straight out of PSUM with the 1/8 logit scale.
"""
import sys

sys.path.insert(0, "/opt/trn_rl_repo")

import numpy as np

import concourse.bacc as bacc
import concourse.tile as tile
from concourse import mybir
from concourse.bass_utils import run_bass_kernel_spmd
from concourse.masks import make_identity

AF = mybir.ActivationFunctionType
F32 = mybir.dt.float32
F32R = mybir.dt.float32r

N_CORES = 8
EMBED_DIM = 1024
NUM_HEADS = 16
HEAD_DIM = 64


def build_attention_core(B, S, D, with_qkv_bias=False):
    """One core's program: 2 heads (E=128 projection slice) of MHA.

    B: batch, S: sequence length per batch, D: model dim.
    Inputs: xT [D, B*S], wq/wk/wv [D, 128], wo [128, D], bq/bk/bv [128].
    Output: out [B*S, D] (partial; host sums over cores).
    """
    P = 128          # partitions / d-chunk / k-tile
    E = 128          # per-core projection width (2 heads x 64)
    HD = 64          # head dim
    QC = 512         # q-chunk (matmul moving free dim)
    BS = B * S
    DC = D // P      # number of contraction chunks for projections
    n_sc = BS // QC  # s-chunks for projections
    n_kt = S // P    # k-tiles per batch
    n_qc = S // QC   # q-chunks per batch
    assert BS % QC == 0 and S % P == 0 and S % QC == 0 and D % P == 0
    scale = 1.0 / np.sqrt(np.float32(HD))

    nc = bacc.Bacc("TRN2", target_bir_lowering=False)
    xT = nc.dram_tensor("xT", [D, BS], F32, kind="ExternalInput")
    wq = nc.dram_tensor("wq", [D, E], F32, kind="ExternalInput")
    wk = nc.dram_tensor("wk", [D, E], F32, kind="ExternalInput")
    wv = nc.dram_tensor("wv", [D, E], F32, kind="ExternalInput")
    wo = nc.dram_tensor("wo", [E, D], F32, kind="ExternalInput")
    bq = nc.dram_tensor("bq", [E], F32, kind="ExternalInput")
    bk = nc.dram_tensor("bk", [E], F32, kind="ExternalInput")
    bv = nc.dram_tensor("bv", [E], F32, kind="ExternalInput")
    out = nc.dram_tensor("out", [BS, D], F32, kind="ExternalOutput")

    xT_r = xT[:].rearrange("(o p) s -> p o s", p=P)      # [P, DC, BS]
    w_r = {n: w[:].rearrange("(o p) e -> p o e", p=P) for n, w in
           (("q", wq), ("k", wk), ("v", wv))}

    with tile.TileContext(nc) as tc:
        with (
            tc.tile_pool(name="persist", bufs=1) as persist,
            tc.tile_pool(name="stage", bufs=2) as stage,
            tc.tile_pool(name="upool", bufs=4) as upool,
            tc.tile_pool(name="small", bufs=3) as small,
            tc.tile_pool(name="outp", bufs=3) as outp,
            tc.tile_pool(name="psA", bufs=2, space="PSUM") as psA,
            tc.tile_pool(name="psB", bufs=4, space="PSUM") as psB,
        ):
            # ---- constants & weights -------------------------------------
            ident32 = persist.tile([P, P], F32)
            make_identity(nc, ident32[:])
            ident = persist.tile([P, P], F32R)
            nc.vector.tensor_copy(ident[:], ident32[:])

            ones32 = persist.tile([P, HD], F32)
            nc.gpsimd.memset(ones32[:], 1.0)
            ones_t = persist.tile([P, HD], F32R)
            nc.vector.tensor_copy(ones_t[:], ones32[:])

            bias_t = {}
            if with_qkv_bias:
                for nm, b in (("q", bq), ("k", bk), ("v", bv)):
                    bs32 = persist.tile([P, 1], F32, tag=f"bias32_{nm}")
                    nc.sync.dma_start(bs32[:], b[:].rearrange("(p o) -> p o", o=1))
                    bt = persist.tile([P, 1], F32R, tag=f"bias_{nm}")
                    nc.vector.tensor_copy(bt[:], bs32[:])
                    bias_t[nm] = bt

            w_t = {}
            for nm in ("q", "k", "v"):
                wstage = stage.tile([P, DC, E], F32, tag="xstage")
                nc.sync.dma_start(wstage[:], w_r[nm])
                wt = persist.tile([P, DC, E], F32R, tag=f"w_{nm}")
                nc.vector.tensor_copy(wt[:], wstage[:])
                w_t[nm] = wt
            wo_stage = stage.tile([E, D], F32, tag="xstage")
            nc.sync.dma_start(wo_stage[:], wo[:])
            wo_t = persist.tile([E, D], F32R)
            nc.vector.tensor_copy(wo_t[:], wo_stage[:])

            # ---- persistent activations ----------------------------------
            QT = persist.tile([P, BS], F32R, tag="QT")     # [e, s]
            KT = persist.tile([P, BS], F32R, tag="KT")     # [e, s]
            # V augmented with ones: per k-chunk [V_h0 | 1 | V_h1 | 1]
            Vaug = persist.tile([P, BS // P, 2 * (HD + 1)], F32R, tag="Vaug")
            nch = BS // P
            assert nch <= HD
            nc.vector.tensor_copy(Vaug[:, :, HD], ones32[:, 0:nch])
            nc.vector.tensor_copy(Vaug[:, :, 2 * HD + 1], ones32[:, 0:nch])

            # ---- phase 1: QT/KT/V projections ----------------------------
            for sc in range(n_sc):
                s0 = sc * QC
                xstage = stage.tile([P, DC, QC], F32, tag="xstage")
                nc.sync.dma_start(xstage[:], xT_r[:, :, s0:s0 + QC])
                xtr = stage.tile([P, DC, QC], F32R, tag="xtr")
                nc.vector.tensor_copy(xtr[:], xstage[:])

                for nm in ("q", "k", "v"):
                    ps = psB.tile([P, QC], F32, tag="B", name=f"ps_{nm}")
                    for o in range(DC):
                        nc.tensor.matmul(
                            ps[:], w_t[nm][:, o, :], xtr[:, o, :],
                            start=(o == 0), stop=(o == DC - 1),
                        )
                    def _bias_add(dst_ap, nm=nm):
                        if with_qkv_bias:
                            nc.vector.tensor_tensor(
                                dst_ap, dst_ap,
                                bias_t[nm][:, 0:1].to_broadcast((P, QC)),
                                mybir.AluOpType.add)
                    if nm == "q":
                        nc.vector.tensor_copy(QT[:, s0:s0 + QC], ps[:])
                        _bias_add(QT[:, s0:s0 + QC])
                    elif nm == "k":
                        nc.vector.tensor_copy(KT[:, s0:s0 + QC], ps[:])
                        _bias_add(KT[:, s0:s0 + QC])
                    else:
                        vt_sb = small.tile([P, QC], F32R, tag="vt")
                        nc.vector.tensor_copy(vt_sb[:], ps[:])
                        _bias_add(vt_sb[:])
                        for ss in range(QC // P):
                            pt = psB.tile([P, P], F32R, tag="B", name="pt")
                            nc.tensor.transpose(
                                pt[:], vt_sb[:, ss * P:(ss + 1) * P], ident[:])
                            ch = sc * (QC // P) + ss
                            nc.vector.tensor_copy(
                                Vaug[:, ch, 0:HD], pt[:, 0:HD])
                            nc.vector.tensor_copy(
                                Vaug[:, ch, HD + 1:2 * HD + 1], pt[:, HD:2 * HD])

            # ---- phase 2: attention + output projection ------------------
            # The softmax tail (reciprocal -> broadcast -> normalize ->
            # output projection) of block i is emitted AFTER block i+1's
            # score/AV stream so its DVE latency chain hides under PE work.
            OCW = min(512, D)

            def emit_ktloop(b, qc):
                q0 = b * S + qc * QC
                pa = [psB.tile([HD + 1, QC], F32, tag="B", name=f"pa{h}")
                      for h in range(2)]
                for kt in range(n_kt):
                    k0 = b * S + kt * P
                    st = psA.tile([P, 2 * QC], F32, tag="A", name="st")
                    nc.tensor.matmul(
                        st[:, 0:QC],
                        KT[0:HD, k0:k0 + P], QT[0:HD, q0:q0 + QC],
                        tile_position=(0, 0), start=True, stop=True)
                    nc.tensor.matmul(
                        st[:, QC:2 * QC],
                        KT[HD:2 * HD, k0:k0 + P], QT[HD:2 * HD, q0:q0 + QC],
                        tile_position=(64, 0), start=True, stop=True)
                    ut = upool.tile([P, 2 * QC], F32R, tag="U")
                    nc.scalar.activation(ut[:], st[:], AF.Exp, scale=float(scale))
                    ch = (b * S) // P + kt
                    for h in range(2):
                        nc.tensor.matmul(
                            pa[h][:],
                            Vaug[:, ch, h * (HD + 1):(h + 1) * (HD + 1)],
                            ut[:, h * QC:(h + 1) * QC],
                            start=(kt == 0), stop=(kt == n_kt - 1))
                return pa

            def emit_tail(b, qc, pa):
                q0 = b * S + qc * QC
                rsb = small.tile([P, 2 * QC], F32R, tag="rsb")
                with nc.allow_low_precision(
                        reason="softmax denominators rounded to f32r "
                               "like every other matmul operand"):
                    for h in range(2):
                        nc.vector.reciprocal(
                            rsb[HD:HD + 1, h * QC:(h + 1) * QC],
                            pa[h][HD:HD + 1, :])
                attnT = small.tile([P, QC], F32R, tag="attnT")
                for h in range(2):
                    prb = psA.tile([HD, QC], F32, tag="A", name="prb")
                    nc.tensor.matmul(
                        prb[:], ones_t[HD:HD + 1, :],
                        rsb[HD:HD + 1, h * QC:(h + 1) * QC],
                        tile_position=(64, 0), start=True, stop=True)
                    prb_sb = small.tile([HD, QC], F32R, tag="prb_sb")
                    nc.vector.tensor_copy(prb_sb[:], prb[:])
                    nc.vector.tensor_tensor(
                        attnT[h * HD:(h + 1) * HD, :],
                        pa[h][0:HD, :], prb_sb[:],
                        mybir.AluOpType.mult)
                for ss in range(QC // P):
                    for oc in range(D // OCW):
                        po = psA.tile([P, OCW], F32, tag="A", name="po")
                        nc.tensor.matmul(
                            po[:], attnT[:, ss * P:(ss + 1) * P],
                            wo_t[:, oc * OCW:(oc + 1) * OCW],
                            start=True, stop=True)
                        osb = outp.tile([P, OCW], F32, tag="osb")
                        nc.vector.tensor_copy(osb[:], po[:])
                        nc.sync.dma_start(
                            out[q0 + ss * P:q0 + (ss + 1) * P,
                                oc * OCW:(oc + 1) * OCW],
                            osb[:])

            blocks = [(b, qc) for b in range(B) for qc in range(n_qc)]
            pending = None   # (b, qc, pa) awaiting its tail
            for b, qc in blocks:
                pa = emit_ktloop(b, qc)
                if pending is not None:
                    emit_tail(*pending)
                pending = (b, qc, pa)
            emit_tail(*pending)

    nc.compile()
    return nc


_NC_CACHE = {}


def _get_nc(B, S, D, with_qkv_bias):
    key = (B, S, D, with_qkv_bias)
    if key not in _NC_CACHE:
        _NC_CACHE[key] = build_attention_core(B, S, D, with_qkv_bias)
    return _NC_CACHE[key]


def run_attention(x, Wq, bq, Wk, bk, Wv, bv, Wo, bo, trace=False):
    B, S, D = x.shape
    with_qkv_bias = bool(np.any(bq) or np.any(bk) or np.any(bv))
    nc = _get_nc(B, S, D, with_qkv_bias)
    xT = np.ascontiguousarray(x.reshape(B * S, D).T)
    in_maps = []
    for c in range(N_CORES):
        sl = slice(c * 128, (c + 1) * 128)
        in_maps.append({
            "xT": xT,
            "wq": np.ascontiguousarray(Wq[:, sl]),
            "wk": np.ascontiguousarray(Wk[:, sl]),
            "wv": np.ascontiguousarray(Wv[:, sl]),
            "wo": np.ascontiguousarray(Wo[sl, :]),
            "bq": np.ascontiguousarray(bq[sl]),
            "bk": np.ascontiguousarray(bk[sl]),
            "bv": np.ascontiguousarray(bv[sl]),
        })
    res = run_bass_kernel_spmd(nc, in_maps, core_ids=list(range(N_CORES)),
                               trace=trace)
    acc = res.results[0]["out"].astype(np.float32).copy()
    for c in range(1, N_CORES):
        acc += res.results[c]["out"]
    acc += bo[None, :]
    return acc.reshape(B, S, D), res


def kernel(x, Wq, bq, Wk, bk, Wv, bv, Wo, bo):
    out, _ = run_attention(np.asarray(x), np.asarray(Wq), np.asarray(bq),
                           np.asarray(Wk), np.asarray(bk), np.asarray(Wv),
                           np.asarray(bv), np.asarray(Wo), np.asarray(bo))
    return out


# revision 18
# speedup vs baseline: 1.3194x; 1.3194x over previous
"""Multi-head self-attention on 8 Trainium2 NeuronCores.

Strategy (tensor parallel over heads, per the classic Megatron split):
  - 16 heads / 8 cores -> each core owns 2 heads (a 128-column slice of
    Wq/Wk/Wv and the matching 128-row slice of Wo).
  - x is transposed on the host to xT [D, B*S] and replicated to every
    core; each core projects QT/KT/VT for its heads, runs attention for
    its (batch, head) pairs, and produces a partial output projection
    [B*S, D].
  - Host sums the 8 partials (the Wo row-parallel all-reduce) and adds bo.

Per-core kernel layout notes:
  - All matmuls run in float32r (full PE rate at free-dim >= 256,
    ~1.5e-4 rel rms per matmul vs fp32).
  - Scores are computed transposed, ST[k, q] = KT.T @ QT, two heads
    row-packed into the PE array (contraction is only 64 wide per head).
  - softmax denominator rides the attention matmul: V is augmented with
    a ones column, so AV psum row 64 is sum_k exp(s).
  - exp happens on ACT straight out of PSUM with the 1/8 logit scale.
"""
import sys

sys.path.insert(0, "/opt/trn_rl_repo")

import numpy as np

import concourse.bacc as bacc
import concourse.tile as tile
from concourse import mybir
from concourse.bass_utils import run_bass_kernel_spmd
from concourse.masks import make_identity

AF = mybir.ActivationFunctionType
F32 = mybir.dt.float32
F32R = mybir.dt.float32r

N_CORES = 8
EMBED_DIM = 1024
NUM_HEADS = 16
HEAD_DIM = 64


def build_attention_core(B, S, D, with_qkv_bias=False):
    """One core's program: 2 heads (E=128 projection slice) of MHA.

    B: batch, S: sequence length per batch, D: model dim.
    Inputs: xT [D, B*S], wq/wk/wv [D, 128], wo [128, D], bq/bk/bv [128].
    Output: out [B*S, D] (partial; host sums over cores).
    """
    P = 128          # partitions / d-chunk / k-tile
    E = 128          # per-core projection width (2 heads x 64)
    HD = 64          # head dim
    QC = 512         # q-chunk (matmul moving free dim)
    BS = B * S
    DC = D // P      # number of contraction chunks for projections
    n_sc = BS // QC  # s-chunks for projections
    n_kt = S // P    # k-tiles per batch
    n_qc = S // QC   # q-chunks per batch
    assert BS % QC == 0 and S % P == 0 and S % QC == 0 and D % P == 0
    scale = 1.0 / np.sqrt(np.float32(HD))

    nc = bacc.Bacc("TRN2", target_bir_lowering=False)
    xT = nc.dram_tensor("xT", [D, BS], F32, kind="ExternalInput")
    wq = nc.dram_tensor("wq", [D, E], F32, kind="ExternalInput")
    wk = nc.dram_tensor("wk", [D, E], F32, kind="ExternalInput")
    wv = nc.dram_tensor("wv", [D, E], F32, kind="ExternalInput")
    wo = nc.dram_tensor("wo", [E, D], F32, kind="ExternalInput")
    bq = nc.dram_tensor("bq", [E], F32, kind="ExternalInput")
    bk = nc.dram_tensor("bk", [E], F32, kind="ExternalInput")
    bv = nc.dram_tensor("bv", [E], F32, kind="ExternalInput")
    out = nc.dram_tensor("out", [BS, D], F32, kind="ExternalOutput")

    xT_r = xT[:].rearrange("(o p) s -> p o s", p=P)      # [P, DC, BS]
    w_r = {n: w[:].rearrange("(o p) e -> p o e", p=P) for n, w in
           (("q", wq), ("k", wk), ("v", wv))}

    with tile.TileContext(nc) as tc:
        with (
            tc.tile_pool(name="persist", bufs=1) as persist,
            tc.tile_pool(name="stage", bufs=2) as stage,
            tc.tile_pool(name="upool", bufs=4) as upool,
            tc.tile_pool(name="small", bufs=3) as small,
            tc.tile_pool(name="outp", bufs=3) as outp,
            tc.tile_pool(name="psA", bufs=2, space="PSUM") as psA,
            tc.tile_pool(name="psB", bufs=4, space="PSUM") as psB,
        ):
            # ---- constants & weights -------------------------------------
            ident32 = persist.tile([P, P], F32)
            make_identity(nc, ident32[:])
            ident = persist.tile([P, P], F32R)
            nc.vector.tensor_copy(ident[:], ident32[:])

            ones32 = persist.tile([P, HD], F32)
            nc.gpsimd.memset(ones32[:], 1.0)
            ones_t = persist.tile([P, HD], F32R)
            nc.vector.tensor_copy(ones_t[:], ones32[:])

            bias_t = {}
            if with_qkv_bias:
                for nm, b in (("q", bq), ("k", bk), ("v", bv)):
                    bs32 = persist.tile([P, 1], F32, tag=f"bias32_{nm}")
                    nc.sync.dma_start(bs32[:], b[:].rearrange("(p o) -> p o", o=1))
                    bt = persist.tile([P, 1], F32R, tag=f"bias_{nm}")
                    nc.vector.tensor_copy(bt[:], bs32[:])
                    bias_t[nm] = bt

            w_t = {}
            for nm in ("q", "k", "v"):
                wstage = stage.tile([P, DC, E], F32, tag="xstage")
                nc.sync.dma_start(wstage[:], w_r[nm])
                wt = persist.tile([P, DC, E], F32R, tag=f"w_{nm}")
                nc.vector.tensor_copy(wt[:], wstage[:])
                w_t[nm] = wt
            wo_stage = stage.tile([E, D], F32, tag="xstage")
            nc.sync.dma_start(wo_stage[:], wo[:])
            wo_t = persist.tile([E, D], F32R)
            nc.vector.tensor_copy(wo_t[:], wo_stage[:])

            # ---- persistent activations ----------------------------------
            QT = persist.tile([P, BS], F32R, tag="QT")     # [e, s]
            KT = persist.tile([P, BS], F32R, tag="KT")     # [e, s]
            # V augmented with ones: per k-chunk [V_h0 | 1 | V_h1 | 1]
            Vaug = persist.tile([P, BS // P, 2 * (HD + 1)], F32R, tag="Vaug")
            nch = BS // P
            assert nch <= HD
            nc.vector.tensor_copy(Vaug[:, :, HD], ones32[:, 0:nch])
            nc.vector.tensor_copy(Vaug[:, :, 2 * HD + 1], ones32[:, 0:nch])

            # ---- phase 1: QT/KT/V projections ----------------------------
            for sc in range(n_sc):
                s0 = sc * QC
                xstage = stage.tile([P, DC, QC], F32, tag="xstage")
                nc.sync.dma_start(xstage[:], xT_r[:, :, s0:s0 + QC])
                xtr = stage.tile([P, DC, QC], F32R, tag="xtr")
                nc.vector.tensor_copy(xtr[:], xstage[:])

                for nm in ("q", "k", "v"):
                    ps = psB.tile([P, QC], F32, tag="B", name=f"ps_{nm}")
                    for o in range(DC):
                        nc.tensor.matmul(
                            ps[:], w_t[nm][:, o, :], xtr[:, o, :],
                            start=(o == 0), stop=(o == DC - 1),
                        )
                    def _bias_add(dst_ap, nm=nm):
                        if with_qkv_bias:
                            nc.vector.tensor_tensor(
                                dst_ap, dst_ap,
                                bias_t[nm][:, 0:1].to_broadcast((P, QC)),
                                mybir.AluOpType.add)
                    if nm == "q":
                        nc.vector.tensor_copy(QT[:, s0:s0 + QC], ps[:])
                        _bias_add(QT[:, s0:s0 + QC])
                    elif nm == "k":
                        nc.vector.tensor_copy(KT[:, s0:s0 + QC], ps[:])
                        _bias_add(KT[:, s0:s0 + QC])
                    else:
                        vt_sb = small.tile([P, QC], F32R, tag="vt")
                        nc.vector.tensor_copy(vt_sb[:], ps[:])
                        _bias_add(vt_sb[:])
                        for ss in range(QC // P):
                            pt = psB.tile([P, P], F32R, tag="B", name="pt")
                            nc.tensor.transpose(
                                pt[:], vt_sb[:, ss * P:(ss + 1) * P], ident[:])
                            ch = sc * (QC // P) + ss
                            nc.vector.tensor_copy(
                                Vaug[:, ch, 0:HD], pt[:, 0:HD])
                            nc.vector.tensor_copy(
                                Vaug[:, ch, HD + 1:2 * HD + 1], pt[:, HD:2 * HD])

            # ---- phase 2: attention + output projection ------------------
            # The softmax tail (reciprocal -> broadcast -> normalize ->
            # output projection) of block i is emitted AFTER block i+1's
            # score/AV stream so its DVE latency chain hides under PE work.
            OCW = min(512, D)

            def emit_ktloop(b, qc):
                q0 = b * S + qc * QC
                pa = [psB.tile([HD + 1, QC], F32, tag="B", name=f"pa{h}")
                      for h in range(2)]
                for kt in range(n_kt):
                    k0 = b * S + kt * P
                    st = psA.tile([P, 2 * QC], F32, tag="A", name="st")
                    nc.tensor.matmul(
                        st[:, 0:QC],
                        KT[0:HD, k0:k0 + P], QT[0:HD, q0:q0 + QC],
                        tile_position=(0, 0), start=True, stop=True)
                    nc.tensor.matmul(
                        st[:, QC:2 * QC],
                        KT[HD:2 * HD, k0:k0 + P], QT[HD:2 * HD, q0:q0 + QC],
                        tile_position=(64, 0), start=True, stop=True)
                    ut = upool.tile([P, 2 * QC], F32R, tag="U")
                    nc.scalar.activation(ut[:], st[:], AF.Exp, scale=float(scale))
                    ch = (b * S) // P + kt
                    for h in range(2):
                        nc.tensor.matmul(
                            pa[h][:],
                            Vaug[:, ch, h * (HD + 1):(h + 1) * (HD + 1)],
                            ut[:, h * QC:(h + 1) * QC],
                            start=(kt == 0), stop=(kt == n_kt - 1))
                return pa

            def emit_tail(b, qc, pa):
                q0 = b * S + qc * QC
                rsb = small.tile([P, 2 * QC], F32R, tag="rsb")
                with nc.allow_low_precision(
                        reason="softmax denominators rounded to f32r "
                               "like every other matmul operand"):
                    for h in range(2):
                        nc.vector.reciprocal(
                            rsb[0:1, h * QC:(h + 1) * QC],
                            pa[h][HD:HD + 1, :])
                attnT = small.tile([P, QC], F32R, tag="attnT")
                for h in range(2):
                    prb_sb = small.tile([HD, QC], F32R, tag="prb_sb")
                    nc.gpsimd.partition_broadcast(
                        prb_sb[:], rsb[0:1, h * QC:(h + 1) * QC])
                    nc.vector.tensor_tensor(
                        attnT[h * HD:(h + 1) * HD, :],
                        pa[h][0:HD, :], prb_sb[:],
                        mybir.AluOpType.mult)
                for ss in range(QC // P):
                    for oc in range(D // OCW):
                        po = psB.tile([P, OCW], F32, tag="B", name="po")
                        nc.tensor.matmul(
                            po[:], attnT[:, ss * P:(ss + 1) * P],
                            wo_t[:, oc * OCW:(oc + 1) * OCW],
                            start=True, stop=True)
                        osb = outp.tile([P, OCW], F32, tag="osb")
                        nc.vector.tensor_copy(osb[:], po[:])
                        nc.sync.dma_start(
                            out[q0 + ss * P:q0 + (ss + 1) * P,
                                oc * OCW:(oc + 1) * OCW],
                            osb[:])

            blocks = [(b, qc) for b in range(B) for qc in range(n_qc)]
            pending = None   # (b, qc, pa) awaiting its tail
            for b, qc in blocks:
                pa = emit_ktloop(b, qc)
                if pending is not None:
                    emit_tail(*pending)
                pending = (b, qc, pa)
            emit_tail(*pending)

    nc.compile()
    return nc


_NC_CACHE = {}


def _get_nc(B, S, D, with_qkv_bias):
    key = (B, S, D, with_qkv_bias)
    if key not in _NC_CACHE:
        _NC_CACHE[key] = build_attention_core(B, S, D, with_qkv_bias)
    return _NC_CACHE[key]


def run_attention(x, Wq, bq, Wk, bk, Wv, bv, Wo, bo, trace=False):
    B, S, D = x.shape
    with_qkv_bias = bool(np.any(bq) or np.any(bk) or np.any(bv))
    nc = _get_nc(B, S, D, with_qkv_bias)
    xT = np.ascontiguousarray(x.reshape(B * S, D).T)
    in_maps = []
    for c in range(N_CORES):
        sl = slice(c * 128, (c + 1) * 128)
        in_maps.append({
            "xT": xT,
            "wq": np.ascontiguousarray(Wq[:, sl]),
            "wk": np.ascontiguousarray(Wk[:, sl]),
            "wv": np.ascontiguousarray(Wv[:, sl]),
            "wo": np.ascontiguousarray(Wo[sl, :]),
            "bq": np.ascontiguousarray(bq[sl]),
            "bk": np.ascontiguousarray(bk[sl]),
            "bv": np.ascontiguousarray(bv[sl]),
        })
    res = run_bass_kernel_spmd(nc, in_maps, core_ids=list(range(N_CORES)),
                               trace=trace)
    acc = res.results[0]["out"].astype(np.float32).copy()
    for c in range(1, N_CORES):
        acc += res.results[c]["out"]
    acc += bo[None, :]
    return acc.reshape(B, S, D), res


def kernel(x, Wq, bq, Wk, bk, Wv, bv, Wo, bo):
    out, _ = run_attention(np.asarray(x), np.asarray(Wq), np.asarray(bq),
                           np.asarray(Wk), np.asarray(bk), np.asarray(Wv),
                           np.asarray(bv), np.asarray(Wo), np.asarray(bo))
    return out
